# revision 40
# baseline (speedup 1.0000x reference)
"""Trainium2 Bass kernel for GQA multi-head attention (B=2, S=2048, H=2048,
32 q heads / 8 kv heads / head_dim 64, RoPE, causal softmax, output proj).

Sharding over 8 NeuronCores: core c handles batch b=c//4 and kv-head pair
j=c%4 (kv heads 2j, 2j+1 -> q heads 8j..8j+7).  Each core computes its
q/k/v projections from a replicated (per-batch) x^T, runs causal attention
for its 8 q heads in a transposed-scores layout (lazy softmax via a
[v|ones] matmul column), and produces a partial output-projection which the
host sums across the 4 cores of each batch.

v3: bf16 matmuls; host-pre-arranged DMA layouts (contiguous slabs);
group-serial projection; software-pipelined attention (scores run ahead of
exp); fine-grained out-projection filler units interleaved into the
attention chunk loop to keep the PE warm; per-tile batched fast-approx
softmax reciprocal, deferred into the next tile's filler stream.
"""

import numpy as np

B, S, H = 2, 2048, 2048
NH, NKV, HD = 32, 8, 64
P = 128
ST = 512           # sequence tile (free dim of most matmuls)
NT = S // ST       # 4 sequence tiles
KC = H // P        # 16 contraction chunks for projections
NCORES = 8

_CACHE = {}


def _build(reps=1, phases='ABC'):
    import concourse.bass as bass
    import concourse.mybir as mybir
    from concourse import bacc
    from concourse.tile import TileContext
    from concourse.masks import make_identity

    f32 = mybir.dt.float32
    f32r = mybir.dt.float32r
    bf16 = mybir.dt.bfloat16
    AF = mybir.ActivationFunctionType
    OP = mybir.AluOpType

    nc = bacc.Bacc("TRN2", target_bir_lowering=False, debug=False,
                   num_devices=NCORES)

    # host-pre-arranged layouts: partition-major contiguous slabs
    xT_d = nc.dram_tensor("xT", [NT * P, KC * ST], bf16, kind="ExternalInput")
    wq_d = nc.dram_tensor("wq", [P, KC * 512], bf16, kind="ExternalInput")
    wk_d = nc.dram_tensor("wk", [P, KC * 128], bf16, kind="ExternalInput")
    wv_d = nc.dram_tensor("wv", [P, KC * 128], bf16, kind="ExternalInput")
    wo_d = nc.dram_tensor("wo", [P, 4 * H], bf16, kind="ExternalInput")
    c2_d = nc.dram_tensor("c2", [P, S], bf16, kind="ExternalInput")
    s2p_d = nc.dram_tensor("s2p", [P, S], bf16, kind="ExternalInput")
    tri_d = nc.dram_tensor("trib", [P, P], bf16, kind="ExternalInput")
    out_d = nc.dram_tensor("out", [S, H], f32, kind="ExternalOutput")

    with TileContext(nc) as tc, \
         nc.allow_low_precision(reason="bf16 kernel, rel-err budget 2e-2"):
        with tc.tile_pool(name="const", bufs=1) as constp, \
             tc.tile_pool(name="qkv", bufs=1) as qkvp, \
             tc.tile_pool(name="attn", bufs=1) as attnp, \
             tc.tile_pool(name="wpool", bufs=1) as wp:

            c2 = constp.tile([P, S], bf16)
            s2p = constp.tile([P, S], bf16)
            tri = constp.tile([P, P], bf16)
            ones_f = constp.tile([P, 1], f32)
            nc.vector.memset(ones_f[:], 1.0)
            ident = constp.tile([64, 64], bf16)
            make_identity(nc, ident[:])
            # sel[:, r, :]: [8, 64] stationary that broadcasts row r of an
            # [8, N] moving operand to 64 output partitions
            ident8 = constp.tile([8, 8], f32)
            make_identity(nc, ident8[:])
            sel = constp.tile([8, 8, 64], f32r)
            for r in range(8):
                nc.vector.tensor_copy(
                    sel[:, r, :], ident8[:, r:r + 1].to_broadcast((8, 64)))
            # dummy exp to pull the ACT table load off the critical path
            dummy = constp.tile([P, 16], f32)
            nc.vector.memset(dummy[:], 0.0)
            nc.scalar.activation(dummy[:], dummy[:], AF.Exp)

            qT = qkvp.tile([P, 4, S], bf16)        # 4 head pairs
            kTd = [qkvp.tile([P, S], bf16, tag=f"ktd{kv}", name=f"ktd{kv}")
                   for kv in range(2)]
            vv = [qkvp.tile([P, KC, 65], bf16, tag=f"v{kv}", name=f"v{kv}")
                  for kv in range(2)]
            attnT = attnp.tile([P, 4, S], bf16)

            # ones column of [v | 1] tiles
            for kv in range(2):
                nc.vector.tensor_copy(
                    vv[kv][:, :, 64:65],
                    ones_f[:, None, 0:1].to_broadcast((P, KC, 1)))

            wq_t = wp.tile([P, KC, 512], bf16)
            wk_t = wp.tile([P, KC, 128], bf16)
            wv_t = wp.tile([P, KC, 128], bf16)
            wo_t = wp.tile([P, 4, H], bf16)

            for rep in range(reps):
                with tc.tile_pool(name="xpool", bufs=2) as xp, \
                     tc.tile_pool(name="ropet", bufs=2) as rp, \
                     tc.tile_pool(name="expp", bufs=4) as ep, \
                     tc.tile_pool(name="nrm", bufs=2) as np_, \
                     tc.tile_pool(name="sbavp", bufs=5) as svp, \
                     tc.tile_pool(name="outp", bufs=2) as outp:

                    if rep == 0:
                        # interleave wq quarters with x quarters on the sync
                        # queue in order of first use; non-critical loads
                        # (c2/s2p tails, tri, wo) are issued later
                        wq_view = wq_d.ap().rearrange("p (k m) -> p k m", k=KC)
                        for c in range(KC):
                            nc.scalar.dma_start(wq_t[:, c], wq_view[:, c])
                        wk_view = wk_d.ap().rearrange("p (k m) -> p k m", k=KC)
                        wv_view = wv_d.ap().rearrange("p (k m) -> p k m", k=KC)
                        for h in range(2):
                            hs = slice(8 * h, 8 * (h + 1))
                            nc.gpsimd.dma_start(wk_t[:, hs], wk_view[:, hs])
                            nc.gpsimd.dma_start(wv_t[:, hs], wv_view[:, hs])
                        nc.gpsimd.dma_start(c2[:, 0:ST], c2_d.ap()[:, 0:ST])
                        nc.gpsimd.dma_start(s2p[:, 0:ST], s2p_d.ap()[:, 0:ST])
                        for h in range(3):
                            hs = slice(ST * (h + 1), ST * (h + 2))
                            nc.gpsimd.dma_start(c2[:, hs], c2_d.ap()[:, hs])
                            nc.gpsimd.dma_start(s2p[:, hs], s2p_d.ap()[:, hs])
                        nc.gpsimd.dma_start(tri[:], tri_d.ap())

                    # ============ Phase A: QKV projection + RoPE ============
                    def rope_into(dst, ps, ts):
                        # dst = ps*c2 + swap(ps*s2p); s2p is pre-swap-permuted
                        t1 = rp.tile([P, ST], f32, tag="t1")
                        nc.vector.tensor_tensor(t1[:], ps[:], c2[:, ts], OP.mult)
                        m2 = rp.tile([P, ST], bf16, tag="m2")
                        nc.vector.tensor_tensor(m2[:], ps[:], s2p[:, ts], OP.mult)
                        m2s = rp.tile([P, ST], bf16, tag="m2s")
                        for b0 in (0, 64):
                            nc.gpsimd.dma_start(m2s[b0:b0 + 32], m2[b0 + 32:b0 + 64])
                            nc.gpsimd.dma_start(m2s[b0 + 32:b0 + 64], m2[b0:b0 + 32])
                        nc.vector.tensor_tensor(dst, t1[:], m2s[:], OP.add)

                    # ---- projection as popable units (interleaved into attention)
                    def proj_units(t):
                        units = []
                        ts = slice(ST * t, ST * (t + 1))
                        pstate = {}

                        def xdma(t=t):
                            if t == 1 and rep == 0:
                                wo_view = wo_d.ap().rearrange(
                                    "p (c e) -> p c e", c=4)
                                for c in range(4):
                                    for h in range(4):
                                        hs = slice(512 * h, 512 * (h + 1))
                                        nc.scalar.dma_start(
                                            wo_t[:, c, hs], wo_view[:, c, hs])
                            xv = xT_d.ap()[P * t:P * (t + 1), :] \
                                .rearrange("p (k s) -> p k s", k=KC)
                            xq = []
                            for qtr in range(4):
                                xt_ = xp.tile([P, 4, ST], bf16, tag=f"xq{qtr}",
                                              name=f"xq{qtr}_{t}")
                                for c in range(4):
                                    nc.sync.dma_start(
                                        xt_[:, c], xv[:, 4 * qtr + c])
                                xq.append(xt_)
                            pstate['xq'] = xq
                        units.append(xdma)

                        for g in range(6):
                            for sub in range(4):
                                def mmu(g=g, sub=sub, t=t):
                                    if sub == 0:
                                        pstate[g] = psO.tile(
                                            [P, ST], f32, tag="po",
                                            name=f"prj{t}_{g}")
                                    ps = pstate[g]
                                    xq = pstate['xq']
                                    for k in range(4 * sub, 4 * sub + 4):
                                        if g < 4:
                                            w_ap = wq_t[:, k, 128 * g:128 * (g + 1)]
                                        elif g == 4:
                                            w_ap = wk_t[:, k]
                                        else:
                                            w_ap = wv_t[:, k]
                                        nc.tensor.matmul(
                                            ps[:], w_ap, xq[k // 4][:, k % 4],
                                            start=(k == 0), stop=(k == KC - 1))
                                units.append(mmu)

                            def drain(g=g, t=t, ts=ts):
                                ps = pstate.pop(g)
                                if g < 4:
                                    rope_into(qT[:, g, ts], ps, ts)
                                elif g == 4:
                                    ktmp = rp.tile([P, ST], bf16, tag="ktmp")
                                    rope_into(ktmp[:], ps, ts)
                                    for kv in range(2):
                                        nc.gpsimd.dma_start(
                                            kTd[kv][0:64, ts],
                                            ktmp[64 * kv:64 * kv + 64])
                                        nc.gpsimd.dma_start(
                                            kTd[kv][64:128, ts],
                                            ktmp[64 * kv:64 * kv + 64])
                                else:
                                    vraw = rp.tile([P, ST], bf16, tag="vraw")
                                    nc.vector.tensor_copy(vraw[:], ps[:])
                                    vraw2 = rp.tile([64, ST], bf16, tag="vraw2")
                                    nc.gpsimd.dma_start(vraw2[:], vraw[64:128])
                                    for blk in range(4):
                                        bs = slice(128 * blk, 128 * (blk + 1))
                                        for kv, vsrc in ((0, vraw), (1, vraw2)):
                                            pst = psO.tile([P, 64], bf16,
                                                           tag="po", name="pst")
                                            nc.tensor.transpose(
                                                pst[:], vsrc[0:64, bs], ident[:])
                                            nc.vector.tensor_copy(
                                                vv[kv][:, 4 * t + blk, 0:64],
                                                pst[:])
                            units.append(drain)
                        return units

                    # ===== Phase B+C: attention + interleaved output proj =====
                    if 'B' not in phases:
                        continue
                    psBC = tc.tile_pool(name="psS", bufs=2, space="PSUM")
                    psS = psBC.__enter__()
                    psVC = tc.tile_pool(name="psV", bufs=2, space="PSUM")
                    psV = psVC.__enter__()
                    psOC = tc.tile_pool(name="psO", bufs=2, space="PSUM")
                    psO = psOC.__enter__()
                    # bootstrap: tiles 0 and 1 projected up-front
                    for u in proj_units(0):
                        u()
                    for u in proj_units(1):
                        u()

                    fillq = []          # fine-grained PE filler closures
                    ot_state = {}

                    def push_outproj(t):
                        # out-projection for token blocks of tile t, as
                        # e-granular units (4 MMs + 1 copy each) + 1 DMA unit
                        for sb in range(4 * t, 4 * (t + 1)):
                            for e in range(4):
                                def u(sb=sb, e=e):
                                    if e == 0:
                                        ot_state[sb] = outp.tile([P, 4, ST], f32,
                                                                 tag="ot",
                                                                 name=f"ot{sb}")
                                    pso = psO.tile([P, ST], f32, tag="po",
                                                   name="pso")
                                    es = slice(ST * e, ST * (e + 1))
                                    for cp in range(4):
                                        nc.tensor.matmul(
                                            pso[:],
                                            attnT[:, cp, P * sb:P * (sb + 1)],
                                            wo_t[:, cp, es],
                                            start=(cp == 0), stop=(cp == 3))
                                    nc.vector.tensor_copy(
                                        ot_state[sb][:, e, :], pso[:])
                                fillq.append(u)
                            def udma(sb=sb):
                                ot = ot_state.pop(sb).rearrange("p e s -> p (e s)")
                                for h in range(4):
                                    ps_ = slice(32 * h, 32 * (h + 1))
                                    nc.sync.dma_start(
                                        out_d.ap()[P * sb + 32 * h:
                                                   P * sb + 32 * (h + 1), :],
                                        ot[ps_])
                            fillq.append(udma)

                    def push_norm(t, sbavs, den_t):
                        # batched softmax normalization for tile t
                        def u():
                            ts = slice(ST * t, ST * (t + 1))
                            rcp_t = np_.tile([8, ST], f32, tag="rcp")
                            nc.vector.reciprocal_approx_fast(rcp_t[:], den_t[:])
                            rcp_r = np_.tile([8, ST], f32r, tag="rcpr")
                            nc.vector.tensor_copy(rcp_r[:], rcp_t[:])
                            for pair in range(4):
                                for par in range(2):
                                    r = 2 * pair + par
                                    psb = psS.tile([P, 2, ST], f32, tag="sc",
                                                   name="psb")
                                    nc.tensor.matmul(psb[0:64, 0, :], sel[:, r, :],
                                                     rcp_r[:])
                                    if par == 0:
                                        nc.vector.tensor_tensor(
                                            attnT[0:64, pair, ts],
                                            sbavs[pair][0:64, 0, :],
                                            psb[0:64, 0, :], OP.mult)
                                    else:
                                        otmp = np_.tile([64, ST], bf16, tag="otmp")
                                        nc.vector.tensor_tensor(
                                            otmp[:], sbavs[pair][0:64, 1, :],
                                            psb[0:64, 0, :], OP.mult)
                                        nc.gpsimd.dma_start(
                                            attnT[64:128, pair, ts], otmp[:])
                        fillq.append(u)

                    def emit_sc(k, t, pair, kv):
                        pss = psS.tile([P, 2, ST], f32, tag="sc", name="pss")
                        c0 = 128 * (k - 4 * t) if k >= 4 * t else 0
                        kk = slice(P * k, P * (k + 1))
                        for par in range(2):
                            p0 = 64 * par
                            nc.tensor.matmul(
                                pss[:, par, c0:],
                                kTd[kv][p0:p0 + 64, kk],
                                qT[p0:p0 + 64, pair, ST * t + c0:ST * (t + 1)])
                        return pss

                    units = [(t, pair) for t in range(NT) for pair in range(4)]
                    carry = []          # pss pre-emitted for the next unit
                    den_by_t = {}
                    sbav_by_t = {}
                    for ui, (t, pair) in enumerate(units):
                        nch = 4 * (t + 1)
                        kv = pair // 2
                        if pair == 0:
                            if t + 2 < NT:
                                fillq.extend(proj_units(t + 2))
                            den_by_t[t] = np_.tile([8, ST], f32, tag="den",
                                                   name=f"den{t}")
                            sbav_by_t[t] = []
                        av2 = [psV.tile([65, ST], f32, tag="av",
                                        name=f"av{ui}_{par}") for par in range(2)]
                        nxt = units[ui + 1] if ui + 1 < len(units) else None
                        q = carry
                        carry = []
                        while len(q) < 2:
                            q.append(emit_sc(len(q), t, pair, kv))
                        for k in range(nch):
                            pss = q[k]
                            c0 = 128 * (k - 4 * t) if k >= 4 * t else 0
                            if k >= 4 * t:
                                nc.vector.tensor_tensor(
                                    pss[:, :, c0:c0 + 128], pss[:, :, c0:c0 + 128],
                                    tri[:, None, :].to_broadcast((P, 2, 128)),
                                    OP.add)
                            ex = ep.tile([P, 2, ST], bf16, tag="ex")
                            nc.scalar.activation(ex[:, :, c0:], pss[:, :, c0:],
                                                 AF.Exp, scale=0.125)
                            if k + 2 < nch:
                                q.append(emit_sc(k + 2, t, pair, kv))
                            elif nxt is not None and len(carry) < 2:
                                carry.append(
                                    emit_sc(len(carry), nxt[0], nxt[1], nxt[1] // 2))
                            for par in range(2):
                                nc.tensor.matmul(av2[par][:, c0:], vv[kv][:, k],
                                                 ex[:, par, c0:],
                                                 start=(k == 0), stop=(k == nch - 1))
                            if fillq:
                                fillq.pop(0)()
                        sbav = svp.tile([65, 2, ST], f32, tag="sbav")
                        for par in range(2):
                            nc.vector.tensor_copy(sbav[:, par, :], av2[par][:])
                        nc.gpsimd.dma_start(
                            den_by_t[t][2 * pair:2 * pair + 2, :], sbav[64:65, :, :])
                        sbav_by_t[t].append(sbav)
                        if fillq:
                            fillq.pop(0)()
                        if pair == 3:
                            push_norm(t, sbav_by_t[t], den_by_t[t])
                            if 'C' in phases:
                                push_outproj(t)

                    while fillq:
                        fillq.pop(0)()
                    psOC.__exit__(None, None, None)
                    psVC.__exit__(None, None, None)
                    psBC.__exit__(None, None, None)

    nc.compile()
    return nc


def _host_prep(x, rotary_cos, rotary_sin, Wq, Wk, Wv, Wo):
    import ml_dtypes
    bf = ml_dtypes.bfloat16
    x = np.asarray(x, np.float32)
    cos = np.asarray(rotary_cos, np.float32)
    sin = np.asarray(rotary_sin, np.float32)
    Wq = np.asarray(Wq, np.float32)
    Wk = np.asarray(Wk, np.float32)
    Wv = np.asarray(Wv, np.float32)
    Wo = np.asarray(Wo, np.float32)

    c2 = np.empty((P, S), np.float32)
    s2p = np.empty((P, S), np.float32)
    for p in range(P):
        c2[p] = cos[:, p % 32]
        s2p[p] = sin[:, p % 32] * (1.0 if (p % 64) < 32 else -1.0)
    c2 = c2.astype(bf)
    s2p = s2p.astype(bf)
    tri = np.where(np.arange(P)[:, None] > np.arange(P)[None, :],
                   np.float32(-1e30), np.float32(0.0)).astype(bf)

    def slab_w(W, nk):
        # [H', M] -> [P, nk*M] where row h = k*P + p
        Hh, M = W.shape
        return np.ascontiguousarray(
            W.reshape(nk, P, M).transpose(1, 0, 2).reshape(P, nk * M)).astype(bf)

    # x: [S, H] -> xT [H, S] -> tiles [NT*P, KC*ST]: tile t row p holds
    # [kc, s] slab of xT[kc*P+p, t*ST:(t+1)*ST]
    xslabs = []
    for b in range(B):
        xT = x[b].T  # [H, S]
        slab = np.empty((NT * P, KC * ST), np.float32)
        for t in range(NT):
            blk = xT[:, ST * t:ST * (t + 1)].reshape(KC, P, ST)
            slab[P * t:P * (t + 1)] = blk.transpose(1, 0, 2).reshape(P, KC * ST)
        xslabs.append(slab.astype(bf))

    in_maps = []
    for c in range(NCORES):
        b, j = divmod(c, 4)
        in_maps.append({
            "xT": xslabs[b],
            "wq": slab_w(Wq[:, 512 * j:512 * (j + 1)], KC),
            "wk": slab_w(Wk[:, 128 * j:128 * (j + 1)], KC),
            "wv": slab_w(Wv[:, 128 * j:128 * (j + 1)], KC),
            "wo": slab_w(Wo[512 * j:512 * (j + 1), :], 4),
            "c2": c2, "s2p": s2p, "trib": tri,
        })
    return in_maps


def kernel(x, rotary_cos, rotary_sin, Wq, Wk, Wv, Wo, reps=1, phases='ABC', _want_res=False):
    from concourse.bass_utils import run_bass_kernel_spmd
    key = (reps, phases)
    if key not in _CACHE:
        _CACHE[key] = _build(reps, phases)
    nc = _CACHE[key]
    in_maps = _host_prep(x, rotary_cos, rotary_sin, Wq, Wk, Wv, Wo)
    res = run_bass_kernel_spmd(nc, in_maps, list(range(NCORES)))
    out = np.empty((B, S, H), np.float32)
    for b in range(B):
        acc = res.results[4 * b]["out"].astype(np.float64)
        for j in range(1, 4):
            acc += res.results[4 * b + j]["out"]
        out[b] = acc.astype(np.float32)
    if _want_res:
        return out, res
    return out


# revision 41
# speedup vs baseline: 1.1966x; 1.1966x over previous
"""Trainium2 Bass kernel for GQA multi-head attention (B=2, S=2048, H=2048,
32 q heads / 8 kv heads / head_dim 64, RoPE, causal softmax, output proj).

Sharding over 8 NeuronCores: core c handles batch b=c//4 and kv-head pair
j=c%4 (kv heads 2j, 2j+1 -> q heads 8j..8j+7).  Each core computes its
q/k/v projections from a replicated (per-batch) x^T, runs causal attention
for its 8 q heads in a transposed-scores layout (lazy softmax via a
[v|ones] matmul column), and produces a partial output-projection which the
host sums across the 4 cores of each batch.

v3: bf16 matmuls; host-pre-arranged DMA layouts (contiguous slabs);
group-serial projection; software-pipelined attention (scores run ahead of
exp); fine-grained out-projection filler units interleaved into the
attention chunk loop to keep the PE warm; per-tile batched fast-approx
softmax reciprocal, deferred into the next tile's filler stream.
"""

import numpy as np

B, S, H = 2, 2048, 2048
NH, NKV, HD = 32, 8, 64
P = 128
ST = 512           # sequence tile (free dim of most matmuls)
NT = S // ST       # 4 sequence tiles
KC = H // P        # 16 contraction chunks for projections
NCORES = 8

_CACHE = {}


def _build(reps=1, phases='ABC'):
    import concourse.bass as bass
    import concourse.mybir as mybir
    from concourse import bacc
    from concourse.tile import TileContext
    from concourse.masks import make_identity

    f32 = mybir.dt.float32
    f32r = mybir.dt.float32r
    bf16 = mybir.dt.bfloat16
    AF = mybir.ActivationFunctionType
    OP = mybir.AluOpType

    nc = bacc.Bacc("TRN2", target_bir_lowering=False, debug=False,
                   num_devices=NCORES)

    # host-pre-arranged layouts: partition-major contiguous slabs
    xT_d = nc.dram_tensor("xT", [NT * P, KC * ST], bf16, kind="ExternalInput")
    wq_d = nc.dram_tensor("wq", [P, KC * 512], bf16, kind="ExternalInput")
    wk_d = nc.dram_tensor("wk", [P, KC * 128], bf16, kind="ExternalInput")
    wv_d = nc.dram_tensor("wv", [P, KC * 128], bf16, kind="ExternalInput")
    wo_d = nc.dram_tensor("wo", [P, 4 * H], bf16, kind="ExternalInput")
    c2_d = nc.dram_tensor("c2", [P, S], bf16, kind="ExternalInput")
    s2p_d = nc.dram_tensor("s2p", [P, S], bf16, kind="ExternalInput")
    tri_d = nc.dram_tensor("trib", [P, P], bf16, kind="ExternalInput")
    out_d = nc.dram_tensor("out", [S, H], f32, kind="ExternalOutput")

    with TileContext(nc) as tc, \
         nc.allow_low_precision(reason="bf16 kernel, rel-err budget 2e-2"):
        with tc.tile_pool(name="const", bufs=1) as constp, \
             tc.tile_pool(name="qkv", bufs=1) as qkvp, \
             tc.tile_pool(name="attn", bufs=1) as attnp, \
             tc.tile_pool(name="wpool", bufs=1) as wp:

            c2 = constp.tile([P, S], bf16)
            s2p = constp.tile([P, S], bf16)
            tri = constp.tile([P, P], bf16)
            ones_f = constp.tile([P, 1], f32)
            nc.vector.memset(ones_f[:], 1.0)
            ident = constp.tile([64, 64], bf16)
            make_identity(nc, ident[:])
            # sel[:, r, :]: [8, 64] stationary that broadcasts row r of an
            # [8, N] moving operand to 64 output partitions
            ident8 = constp.tile([8, 8], f32)
            make_identity(nc, ident8[:])
            sel = constp.tile([8, 8, 64], f32r)
            for r in range(8):
                nc.vector.tensor_copy(
                    sel[:, r, :], ident8[:, r:r + 1].to_broadcast((8, 64)))
            # dummy exp to pull the ACT table load off the critical path
            dummy = constp.tile([P, 16], f32)
            nc.vector.memset(dummy[:], 0.0)
            nc.scalar.activation(dummy[:], dummy[:], AF.Exp)

            qT = qkvp.tile([P, 4, S], bf16)        # 4 head pairs
            kTd = [qkvp.tile([P, S], bf16, tag=f"ktd{kv}", name=f"ktd{kv}")
                   for kv in range(2)]
            vv = [qkvp.tile([P, KC, 65], bf16, tag=f"v{kv}", name=f"v{kv}")
                  for kv in range(2)]
            attnT = attnp.tile([P, 4, S], bf16)

            # ones column of [v | 1] tiles
            for kv in range(2):
                nc.vector.tensor_copy(
                    vv[kv][:, :, 64:65],
                    ones_f[:, None, 0:1].to_broadcast((P, KC, 1)))

            wq_t = wp.tile([P, KC, 512], bf16)
            wk_t = wp.tile([P, KC, 128], bf16)
            wv_t = wp.tile([P, KC, 128], bf16)
            wo_t = wp.tile([P, 4, H], bf16)

            for rep in range(reps):
                with tc.tile_pool(name="xpool", bufs=2) as xp, \
                     tc.tile_pool(name="ropet", bufs=2) as rp, \
                     tc.tile_pool(name="expp", bufs=4) as ep, \
                     tc.tile_pool(name="nrm", bufs=2) as np_, \
                     tc.tile_pool(name="sbavp", bufs=4) as svp, \
                     tc.tile_pool(name="outp", bufs=2) as outp:

                    if rep == 0:
                        # interleave wq quarters with x quarters on the sync
                        # queue in order of first use; non-critical loads
                        # (c2/s2p tails, tri, wo) are issued later
                        wq_view = wq_d.ap().rearrange("p (k m) -> p k m", k=KC)
                        for c in range(KC):
                            nc.scalar.dma_start(wq_t[:, c], wq_view[:, c])
                        wk_view = wk_d.ap().rearrange("p (k m) -> p k m", k=KC)
                        wv_view = wv_d.ap().rearrange("p (k m) -> p k m", k=KC)
                        for h in range(2):
                            hs = slice(8 * h, 8 * (h + 1))
                            nc.gpsimd.dma_start(wk_t[:, hs], wk_view[:, hs])
                            nc.gpsimd.dma_start(wv_t[:, hs], wv_view[:, hs])
                        nc.gpsimd.dma_start(c2[:, 0:ST], c2_d.ap()[:, 0:ST])
                        nc.gpsimd.dma_start(s2p[:, 0:ST], s2p_d.ap()[:, 0:ST])
                        for h in range(3):
                            hs = slice(ST * (h + 1), ST * (h + 2))
                            nc.gpsimd.dma_start(c2[:, hs], c2_d.ap()[:, hs])
                            nc.gpsimd.dma_start(s2p[:, hs], s2p_d.ap()[:, hs])
                        nc.gpsimd.dma_start(tri[:], tri_d.ap())

                    # ============ Phase A: QKV projection + RoPE ============
                    def rope_into(dst, ps, ts):
                        # dst = ps*c2 + swap(ps*s2p); s2p is pre-swap-permuted
                        t1 = rp.tile([P, ST], f32, tag="t1")
                        nc.vector.tensor_tensor(t1[:], ps[:], c2[:, ts], OP.mult)
                        m2 = rp.tile([P, ST], bf16, tag="m2")
                        nc.vector.tensor_tensor(m2[:], ps[:], s2p[:, ts], OP.mult)
                        m2s = rp.tile([P, ST], bf16, tag="m2s")
                        for b0 in (0, 64):
                            nc.gpsimd.dma_start(m2s[b0:b0 + 32], m2[b0 + 32:b0 + 64])
                            nc.gpsimd.dma_start(m2s[b0 + 32:b0 + 64], m2[b0:b0 + 32])
                        nc.vector.tensor_tensor(dst, t1[:], m2s[:], OP.add)

                    if 'A' in phases:
                      with tc.tile_pool(name="psA", bufs=6, space="PSUM") as psA, \
                           tc.tile_pool(name="psT", bufs=2, space="PSUM") as psT:
                        for t in range(NT):
                            if t == 1 and rep == 0:
                                wo_view = wo_d.ap().rearrange(
                                    "p (c e) -> p c e", c=4)
                                for c in range(4):
                                    for h in range(4):
                                        hs = slice(512 * h, 512 * (h + 1))
                                        nc.scalar.dma_start(
                                            wo_t[:, c, hs], wo_view[:, c, hs])
                            ts = slice(ST * t, ST * (t + 1))
                            xv = xT_d.ap()[P * t:P * (t + 1), :] \
                                .rearrange("p (k s) -> p k s", k=KC)
                            # quarter tiles so the first matmuls only gate on
                            # the first 512KB of the tile's x slab
                            xq = []
                            for qtr in range(4):
                                xt_ = xp.tile([P, 4, ST], bf16, tag=f"xq{qtr}",
                                              name=f"xq{qtr}_{t}")
                                for c in range(4):
                                    nc.sync.dma_start(
                                        xt_[:, c], xv[:, 4 * qtr + c])
                                xq.append(xt_)
                            for g in range(6):
                                ps = psA.tile([P, ST], f32, tag="proj")
                                for k in range(KC):
                                    if g < 4:
                                        w_ap = wq_t[:, k, 128 * g:128 * (g + 1)]
                                    elif g == 4:
                                        w_ap = wk_t[:, k]
                                    else:
                                        w_ap = wv_t[:, k]
                                    nc.tensor.matmul(ps[:], w_ap,
                                                     xq[k // 4][:, k % 4],
                                                     start=(k == 0), stop=(k == KC - 1))
                                if g < 4:
                                    rope_into(qT[:, g, ts], ps, ts)
                                elif g == 4:
                                    ktmp = rp.tile([P, ST], bf16, tag="ktmp")
                                    rope_into(ktmp[:], ps, ts)
                                    for kv in range(2):
                                        nc.gpsimd.dma_start(
                                            kTd[kv][0:64, ts], ktmp[64 * kv:64 * kv + 64])
                                        nc.gpsimd.dma_start(
                                            kTd[kv][64:128, ts], ktmp[64 * kv:64 * kv + 64])
                                else:
                                    vraw = rp.tile([P, ST], bf16, tag="vraw")
                                    nc.vector.tensor_copy(vraw[:], ps[:])
                                    vraw2 = rp.tile([64, ST], bf16, tag="vraw2")
                                    nc.gpsimd.dma_start(vraw2[:], vraw[64:128])
                                    for blk in range(4):
                                        bs = slice(128 * blk, 128 * (blk + 1))
                                        for kv, vsrc in ((0, vraw), (1, vraw2)):
                                            pst = psT.tile([P, 64], bf16, tag="pst")
                                            nc.tensor.transpose(pst[:], vsrc[0:64, bs],
                                                                ident[:])
                                            nc.vector.tensor_copy(
                                                vv[kv][:, 4 * t + blk, 0:64], pst[:])

                    # ===== Phase B+C: attention + interleaved output proj =====
                    if 'B' not in phases:
                        continue
                    psBC = tc.tile_pool(name="psS", bufs=3, space="PSUM")
                    psS = psBC.__enter__()
                    psVC = tc.tile_pool(name="psV", bufs=1, space="PSUM")
                    psV = psVC.__enter__()

                    fillq = []          # fine-grained PE filler closures
                    ot_state = {}

                    def push_outproj(t):
                        # out-projection for token blocks of tile t, as
                        # e-granular units (4 MMs + 1 copy each) + 1 DMA unit
                        for sb in range(4 * t, 4 * (t + 1)):
                            for e in range(4):
                                def u(sb=sb, e=e):
                                    if e == 0:
                                        ot_state[sb] = outp.tile([P, 4, ST], f32,
                                                                 tag="ot",
                                                                 name=f"ot{sb}")
                                    pso = psS.tile([P, 2, ST], f32, tag="sc",
                                                   name="pso")
                                    es = slice(ST * e, ST * (e + 1))
                                    for cp in range(4):
                                        nc.tensor.matmul(
                                            pso[:, 0, :],
                                            attnT[:, cp, P * sb:P * (sb + 1)],
                                            wo_t[:, cp, es],
                                            start=(cp == 0), stop=(cp == 3))
                                    nc.vector.tensor_copy(
                                        ot_state[sb][:, e, :], pso[:, 0, :])
                                fillq.append(u)
                            def udma(sb=sb):
                                ot = ot_state.pop(sb).rearrange("p e s -> p (e s)")
                                for h in range(4):
                                    ps_ = slice(32 * h, 32 * (h + 1))
                                    nc.sync.dma_start(
                                        out_d.ap()[P * sb + 32 * h:
                                                   P * sb + 32 * (h + 1), :],
                                        ot[ps_])
                            fillq.append(udma)

                    def push_norm(t, sbavs, den_t):
                        # batched softmax normalization for tile t
                        def u():
                            ts = slice(ST * t, ST * (t + 1))
                            rcp_t = np_.tile([8, ST], f32, tag="rcp")
                            nc.vector.reciprocal_approx_fast(rcp_t[:], den_t[:])
                            rcp_r = np_.tile([8, ST], f32r, tag="rcpr")
                            nc.vector.tensor_copy(rcp_r[:], rcp_t[:])
                            for pair in range(4):
                                for par in range(2):
                                    r = 2 * pair + par
                                    psb = psS.tile([P, 2, ST], f32, tag="sc",
                                                   name="psb")
                                    nc.tensor.matmul(psb[0:64, 0, :], sel[:, r, :],
                                                     rcp_r[:])
                                    if par == 0:
                                        nc.vector.tensor_tensor(
                                            attnT[0:64, pair, ts],
                                            sbavs[pair][0:64, 0, :],
                                            psb[0:64, 0, :], OP.mult)
                                    else:
                                        otmp = np_.tile([64, ST], bf16, tag="otmp")
                                        nc.vector.tensor_tensor(
                                            otmp[:], sbavs[pair][0:64, 1, :],
                                            psb[0:64, 0, :], OP.mult)
                                        nc.gpsimd.dma_start(
                                            attnT[64:128, pair, ts], otmp[:])
                        fillq.append(u)

                    def emit_sc(k, t, pair, kv):
                        pss = psS.tile([P, 2, ST], f32, tag="sc", name="pss")
                        c0 = 128 * (k - 4 * t) if k >= 4 * t else 0
                        kk = slice(P * k, P * (k + 1))
                        for par in range(2):
                            p0 = 64 * par
                            nc.tensor.matmul(
                                pss[:, par, c0:],
                                kTd[kv][p0:p0 + 64, kk],
                                qT[p0:p0 + 64, pair, ST * t + c0:ST * (t + 1)])
                        return pss

                    units = [(t, pair) for t in range(NT) for pair in range(4)]
                    carry = []          # pss pre-emitted for the next unit
                    den_by_t = {}
                    sbav_by_t = {}
                    for ui, (t, pair) in enumerate(units):
                        nch = 4 * (t + 1)
                        kv = pair // 2
                        if pair == 0:
                            den_by_t[t] = np_.tile([8, ST], f32, tag="den",
                                                   name=f"den{t}")
                            sbav_by_t[t] = []
                        av = psV.tile([65, 2, ST], f32, tag="av")
                        nxt = units[ui + 1] if ui + 1 < len(units) else None
                        q = carry
                        carry = []
                        while len(q) < 2:
                            q.append(emit_sc(len(q), t, pair, kv))
                        for k in range(nch):
                            pss = q[k]
                            c0 = 128 * (k - 4 * t) if k >= 4 * t else 0
                            if k >= 4 * t:
                                nc.vector.tensor_tensor(
                                    pss[:, :, c0:c0 + 128], pss[:, :, c0:c0 + 128],
                                    tri[:, None, :].to_broadcast((P, 2, 128)),
                                    OP.add)
                            ex = ep.tile([P, 2, ST], bf16, tag="ex")
                            nc.scalar.activation(ex[:, :, c0:], pss[:, :, c0:],
                                                 AF.Exp, scale=0.125)
                            if k + 2 < nch:
                                q.append(emit_sc(k + 2, t, pair, kv))
                            elif nxt is not None and len(carry) < 2:
                                carry.append(
                                    emit_sc(len(carry), nxt[0], nxt[1], nxt[1] // 2))
                            for par in range(2):
                                nc.tensor.matmul(av[:, par, c0:], vv[kv][:, k],
                                                 ex[:, par, c0:],
                                                 start=(k == 0), stop=(k == nch - 1))
                            if fillq:
                                fillq.pop(0)()
                        sbav = svp.tile([65, 2, ST], f32, tag="sbav")
                        nc.vector.tensor_copy(sbav[:], av[:])
                        nc.gpsimd.dma_start(
                            den_by_t[t][2 * pair:2 * pair + 2, :], sbav[64:65, :, :])
                        sbav_by_t[t].append(sbav)
                        if fillq:
                            fillq.pop(0)()
                        if pair == 3:
                            push_norm(t, sbav_by_t[t], den_by_t[t])
                            if 'C' in phases:
                                push_outproj(t)

                    while fillq:
                        fillq.pop(0)()
                    psVC.__exit__(None, None, None)
                    psBC.__exit__(None, None, None)

    nc.compile()
    return nc


def _host_prep(x, rotary_cos, rotary_sin, Wq, Wk, Wv, Wo):
    import ml_dtypes
    bf = ml_dtypes.bfloat16
    x = np.asarray(x, np.float32)
    cos = np.asarray(rotary_cos, np.float32)
    sin = np.asarray(rotary_sin, np.float32)
    Wq = np.asarray(Wq, np.float32)
    Wk = np.asarray(Wk, np.float32)
    Wv = np.asarray(Wv, np.float32)
    Wo = np.asarray(Wo, np.float32)

    c2 = np.empty((P, S), np.float32)
    s2p = np.empty((P, S), np.float32)
    for p in range(P):
        c2[p] = cos[:, p % 32]
        s2p[p] = sin[:, p % 32] * (1.0 if (p % 64) < 32 else -1.0)
    c2 = c2.astype(bf)
    s2p = s2p.astype(bf)
    tri = np.where(np.arange(P)[:, None] > np.arange(P)[None, :],
                   np.float32(-1e30), np.float32(0.0)).astype(bf)

    def slab_w(W, nk):
        # [H', M] -> [P, nk*M] where row h = k*P + p
        Hh, M = W.shape
        return np.ascontiguousarray(
            W.reshape(nk, P, M).transpose(1, 0, 2).reshape(P, nk * M)).astype(bf)

    # x: [S, H] -> xT [H, S] -> tiles [NT*P, KC*ST]: tile t row p holds
    # [kc, s] slab of xT[kc*P+p, t*ST:(t+1)*ST]
    xslabs = []
    for b in range(B):
        xT = x[b].T  # [H, S]
        slab = np.empty((NT * P, KC * ST), np.float32)
        for t in range(NT):
            blk = xT[:, ST * t:ST * (t + 1)].reshape(KC, P, ST)
            slab[P * t:P * (t + 1)] = blk.transpose(1, 0, 2).reshape(P, KC * ST)
        xslabs.append(slab.astype(bf))

    in_maps = []
    for c in range(NCORES):
        b, j = divmod(c, 4)
        in_maps.append({
            "xT": xslabs[b],
            "wq": slab_w(Wq[:, 512 * j:512 * (j + 1)], KC),
            "wk": slab_w(Wk[:, 128 * j:128 * (j + 1)], KC),
            "wv": slab_w(Wv[:, 128 * j:128 * (j + 1)], KC),
            "wo": slab_w(Wo[512 * j:512 * (j + 1), :], 4),
            "c2": c2, "s2p": s2p, "trib": tri,
        })
    return in_maps


def kernel(x, rotary_cos, rotary_sin, Wq, Wk, Wv, Wo, reps=1, phases='ABC', _want_res=False):
    from concourse.bass_utils import run_bass_kernel_spmd
    key = (reps, phases)
    if key not in _CACHE:
        _CACHE[key] = _build(reps, phases)
    nc = _CACHE[key]
    in_maps = _host_prep(x, rotary_cos, rotary_sin, Wq, Wk, Wv, Wo)
    res = run_bass_kernel_spmd(nc, in_maps, list(range(NCORES)))
    out = np.empty((B, S, H), np.float32)
    for b in range(B):
        acc = res.results[4 * b]["out"].astype(np.float64)
        for j in range(1, 4):
            acc += res.results[4 * b + j]["out"]
        out[b] = acc.astype(np.float32)
    if _want_res:
        return out, res
    return out
